# revision 13
# baseline (speedup 1.0000x reference)
"""NSF_CL (neural spline flow coupling layers) fused Bass kernel for TRN2.

Pure data parallel over 8 NeuronCores (4096 samples each), fully fused
on-chip per 512-sample tile:
  - PE-transpose x to channel-major [c, s]
  - conditioner MLP with error-compensated f32r matmuls:
      u = rh@wh  (f32r, exact products)
        + rl@wh  (f32r: rl = x - rh, exact)
        + bf16(rh)@bf16(w - wh)   (weight-rounding correction)
    applied to layer 1 and the Wu/Hu output columns (the spline width /
    height params are ~1000x error-amplifying); Du columns single-term.
  - host-permuted layer-2 weights make each output tile one spline
    parameter plane [channel, sample]
  - rational-quadratic spline via DVE/ACT ops; the bin gather is a
    copy_predicated chain over cumulative-softmax planes (u8 masks)
  - log_det = ones-vector matmul over channels
  - PE-transpose back, DMA z / log_det out.
"""
import numpy as np

import concourse.bass as bass
import concourse.mybir as mybir
import concourse.tile as tile
from concourse import bacc
from concourse.bass_utils import run_bass_kernel_spmd
from concourse.masks import make_identity

F32 = mybir.dt.float32
F32R = mybir.dt.float32r
BF16 = mybir.dt.bfloat16
U8 = mybir.dt.uint8
AF = mybir.ActivationFunctionType
OP = mybir.AluOpType

K = 5
B = 3.0
MIN_BW = 1e-3
MIN_D = 1e-3
ALPHA = 2.0 * B * (1.0 - K * MIN_BW)
BETA = 2.0 * B * MIN_BW
SP2_EDGE = 1.0 - MIN_D

N = 32768
DIM = 256
HALF = 128
HID = 512
NPAR = 14
NCOMP = 10           # compensated param columns (Wu+Hu)
NCORES = 8
NS = 512
NT = (N // NCORES) // NS
NCORE = N // NCORES


class Ctx:
    pass


def _st(c, nm, dtype=F32, shape=None):
    return c.sb.tile(shape or [128, NS], dtype, tag="stmp", bufs=8, name=nm)


def _emit_mlp(c, W, rhs_r, rhs_rl, rhs_hb):
    """Compensated MLP.  W = dict(w0h, w0lb, w1h, w1lb).
    rhs_r: f32r main rhs; rhs_rl: f32r low part; rhs_hb: bf16 of rhs_r."""
    nc = c.nc
    h_r = c.sb.tile([128, 4, NS], F32R, tag="h_r", name="h_r")
    h_rl = c.sb.tile([128, 4, NS], F32R, tag="h_rl", name="h_rl")
    h_hb = c.sb.tile([128, 4, NS], BF16, tag="h_hb", name="h_hb")
    for hc in range(4):
        sl = slice(hc * 128, (hc + 1) * 128)
        psh = c.ps_h.tile([128, NS], F32, tag="psh", name="psh")
        nc.tensor.matmul(psh[:], W["w0h"][:, sl], rhs_r, start=True, stop=False)
        nc.tensor.matmul(psh[:], W["w0h"][:, sl], rhs_rl, start=False, stop=False)
        nc.tensor.matmul(psh[:], W["w0lb"][:, sl], rhs_hb, start=False, stop=True)
        # tanh(a) = 1 - 2/(exp(2a)+1)
        e2 = _st(c, "e2")
        nc.scalar.activation(e2[:], psh[:], AF.Exp, scale=2.0)
        den = _st(c, "e2den")
        nc.vector.tensor_scalar(den[:], e2[:], 1.0, 1e30, OP.add, OP.min)
        rec = _st(c, "e2rec")
        nc.vector.reciprocal_approx_fast(rec[:], den[:])
        h32 = _st(c, "h32")
        nc.vector.tensor_scalar(h32[:], rec[:], -2.0, 1.0, OP.mult, OP.add)
        nc.scalar.copy(h_r[:, hc, :], h32[:])
        nc.vector.tensor_tensor(h_rl[:, hc, :], h32[:], h_r[:, hc, :],
                                OP.subtract)
        nc.scalar.copy(h_hb[:, hc, :], h_r[:, hc, :])

    def group(gi):
        pbase, np_, comp = ((0, 5, True), (5, 5, True), (10, 4, False))[gi]
        psp = c.ps_par.tile([128, 5, NS], F32, tag="pspar", name="pspar")
        for p in range(np_):
            pc = pbase + p
            for hc in range(4):
                last = (not comp) and hc == 3
                nc.tensor.matmul(
                    psp[:, p, :], W["w1h"][:, hc, pc * 128:(pc + 1) * 128],
                    h_r[:, hc, :], start=(hc == 0), stop=last)
                if comp:
                    nc.tensor.matmul(
                        psp[:, p, :], W["w1h"][:, hc, pc * 128:(pc + 1) * 128],
                        h_rl[:, hc, :], start=False, stop=False)
            if comp:
                for hc in range(4):
                    nc.tensor.matmul(
                        psp[:, p, :], W["w1lb"][:, hc, pc * 128:(pc + 1) * 128],
                        h_hb[:, hc, :], start=False, stop=(hc == 3))
        et = c.sb.tile([128, np_, NS], F32, tag="expbuf", bufs=2,
                       name=f"exp_g{gi}")
        nc.scalar.activation(et[:], psp[:, 0:np_, :], AF.Exp)
        return et
    return group


def _emit_half(c, ew, jslot):
    nc = c.nc
    sw = _st(c, "sw")
    nc.vector.tensor_tensor(sw[:], ew[:, 0, :], ew[:, 1, :], OP.add)
    nc.vector.tensor_tensor(sw[:], sw[:], ew[:, 2, :], OP.add)
    nc.vector.tensor_tensor(sw[:], sw[:], ew[:, 3, :], OP.add)
    nc.vector.tensor_tensor(sw[:], sw[:], ew[:, 4, :], OP.add)
    rw = _st(c, "rw")
    nc.vector.reciprocal_approx_fast(rw[:], sw[:])
    nc.vector.tensor_tensor(
        ew[:], ew[:], rw[:].unsqueeze(1).to_broadcast([128, 5, NS]), OP.mult)
    ew2 = c.sb.tile([128, 5, NS], F32, tag="expbuf", bufs=2, name="ew2")
    nc.scalar.activation(ew2[:], ew[:], AF.Exp, scale=2.0 * B)
    pp = c.sb.tile([128, 4, NS], F32, tag="bigbuf", name="pp")
    nc.vector.tensor_copy(pp[:, 0, :], ew2[:, 0, :])
    for k in range(1, 4):
        nc.vector.tensor_tensor(pp[:, k, :], pp[:, k - 1, :], ew2[:, k, :],
                                OP.add)
    s2 = _st(c, "s2")
    nc.vector.tensor_tensor(s2[:], pp[:, 3, :], ew2[:, 4, :], OP.add)
    r2 = _st(c, "r2")
    nc.vector.reciprocal_approx_fast(r2[:], s2[:])
    nc.vector.tensor_tensor(
        c.S[:, jslot, 0:4, :], pp[:],
        r2[:].unsqueeze(1).to_broadcast([128, 4, NS]), OP.mult)


def _emit_coupling(c, W, x_sp, x_sp_r, x_sp_rl, x_sp_hb, y, need_yr):
    nc = c.nc
    group = _emit_mlp(c, W, x_sp_r, x_sp_rl, x_sp_hb)

    ew = group(0)
    _emit_half(c, ew, 0)
    eh = group(1)
    _emit_half(c, eh, 1)
    ed = group(2)
    nc.scalar.activation(c.S[:, 2, 0:4, :], ed[:], AF.Ln, bias=c.consts[2.0][:])

    xc = _st(c, "xc")
    nc.vector.tensor_scalar(xc[:], x_sp, B, -B, OP.min, OP.max)
    nc.vector.tensor_tensor(c.outm[:], x_sp, xc[:], OP.not_equal)
    xcb = _st(c, "xcb")
    nc.scalar.activation(xcb[:], xc[:], AF.Identity, bias=c.consts[B][:])
    lhs = c.sb.tile([128, 4, NS], F32, tag="bigbuf", name="lhs")
    for k in range(1, 5):
        nc.vector.tensor_scalar(lhs[:, k - 1, :], xc[:], 1.0 / ALPHA,
                                (B - BETA * k) / ALPHA, OP.mult, OP.add)
    nc.vector.tensor_tensor(c.msk[:], lhs[:], c.S[:, 0, 0:4, :], OP.is_ge)
    idx_u = _st(c, "idx_u", U8)
    nc.vector.tensor_tensor(idx_u[:], c.msk[:, 0, :], c.msk[:, 1, :], OP.add)
    nc.vector.tensor_tensor(idx_u[:], idx_u[:], c.msk[:, 2, :], OP.add)
    nc.vector.tensor_tensor(idx_u[:], idx_u[:], c.msk[:, 3, :], OP.add)
    idx = c.ll["idx"]
    nc.vector.tensor_copy(idx[:], idx_u[:])

    G = c.sb.tile([128, 3, 2, NS], F32, tag="bigbuf", name="G")
    nc.vector.memset(G[:, 0:2, 0, :], 0.0)
    nc.vector.memset(G[:, 2, 0, :], SP2_EDGE)
    nc.vector.tensor_copy(G[:, :, 1, :], c.S[:, :, 0, :])
    for k in range(1, 5):
        nc.vector.copy_predicated(
            G[:],
            c.msk[:, k - 1, :].unsqueeze(1).unsqueeze(1)
                .to_broadcast([128, 3, 2, NS]),
            c.S[:, :, k - 1:k + 1, :])
    G0m, G1m = G[:, 0, 0, :], G[:, 0, 1, :]
    G0h, G1h = G[:, 1, 0, :], G[:, 1, 1, :]
    G0d, G1d = G[:, 2, 0, :], G[:, 2, 1, :]

    iw0 = _st(c, "iw0")
    nc.vector.tensor_tensor(iw0[:], G1m, G0m, OP.subtract)
    inw = _st(c, "inw")
    nc.scalar.activation(inw[:], iw0[:], AF.Identity, bias=c.consts[BETA][:],
                         scale=ALPHA)
    rinw = _st(c, "rinw")
    nc.vector.reciprocal_approx_fast(rinw[:], inw[:])
    ih0 = _st(c, "ih0")
    nc.vector.tensor_tensor(ih0[:], G1h, G0h, OP.subtract)
    inh = c.ll["inh"]
    nc.scalar.activation(inh[:], ih0[:], AF.Identity, bias=c.consts[BETA][:],
                         scale=ALPHA)
    t0 = _st(c, "t0")
    nc.vector.scalar_tensor_tensor(t0[:], idx[:], -BETA, xcb[:], OP.mult, OP.add)
    t1 = _st(c, "t1")
    nc.vector.scalar_tensor_tensor(t1[:], G0m, -ALPHA, t0[:], OP.mult, OP.add)
    theta = _st(c, "theta")
    nc.vector.tensor_tensor(theta[:], t1[:], rinw[:], OP.mult)
    delta = c.ll["delta"]
    nc.vector.tensor_tensor(delta[:], inh[:], rinw[:], OP.mult)
    onemt = _st(c, "onemt")
    nc.scalar.activation(onemt[:], theta[:], AF.Identity, bias=c.consts[1.0][:],
                         scale=-1.0)
    th2 = _st(c, "th2")
    nc.scalar.square(th2[:], theta[:])
    omt2 = _st(c, "omt2")
    nc.scalar.square(omt2[:], onemt[:])
    t1m = c.ll["t1m"]
    nc.vector.tensor_tensor(t1m[:], theta[:], onemt[:], OP.mult)
    u1 = _st(c, "u1")
    nc.vector.scalar_tensor_tensor(u1[:], G1d, MIN_D, th2[:], OP.add, OP.mult)
    u2 = _st(c, "u2")
    nc.vector.tensor_tensor(u2[:], delta[:], t1m[:], OP.mult)
    u3 = _st(c, "u3")
    nc.vector.scalar_tensor_tensor(u3[:], G0d, MIN_D, omt2[:], OP.add, OP.mult)
    a2 = _st(c, "a2")
    nc.vector.tensor_tensor(a2[:], delta[:], th2[:], OP.mult)
    b2 = _st(c, "b2")
    nc.vector.scalar_tensor_tensor(b2[:], G0d, MIN_D, t1m[:], OP.add, OP.mult)
    ns_ = _st(c, "ns")
    nc.vector.tensor_tensor(ns_[:], a2[:], b2[:], OP.add)
    dn = _st(c, "dn")
    nc.vector.scalar_tensor_tensor(dn[:], u2[:], 2.0, u1[:], OP.mult, OP.add)
    nc.vector.tensor_tensor(dn[:], dn[:], u3[:], OP.add)
    l2 = c.ll["l2"]
    nc.scalar.activation(l2[:], dn[:], AF.Ln)
    num = _st(c, "num")
    nc.vector.tensor_tensor(num[:], ns_[:], inh[:], OP.mult)
    sdd = _st(c, "sdd")
    nc.vector.tensor_tensor(sdd[:], G0d, G1d, OP.add)
    sm = _st(c, "sm")
    nc.vector.scalar_tensor_tensor(sm[:], delta[:], -2.0, sdd[:], OP.mult, OP.add)
    smt = _st(c, "smt")
    nc.vector.scalar_tensor_tensor(smt[:], sm[:], 2.0 * MIN_D, t1m[:],
                                   OP.add, OP.mult)
    den = _st(c, "den")
    nc.vector.tensor_tensor(den[:], smt[:], delta[:], OP.add)
    rd = _st(c, "rd")
    nc.vector.reciprocal_approx_fast(rd[:], den[:])
    q = _st(c, "q")
    nc.vector.tensor_tensor(q[:], num[:], rd[:], OP.mult)
    ty = _st(c, "ty")
    nc.vector.scalar_tensor_tensor(ty[:], G0h, ALPHA, q[:], OP.mult, OP.add)
    y0t = _st(c, "y0t")
    nc.vector.scalar_tensor_tensor(y0t[:], idx[:], BETA, ty[:], OP.mult, OP.add)
    nc.scalar.activation(y[:], y0t[:], AF.Identity, bias=c.consts[-B][:])
    drd = _st(c, "drd")
    nc.vector.tensor_tensor(drd[:], delta[:], rd[:], OP.mult)
    l1 = _st(c, "l1")
    nc.scalar.activation(l1[:], drd[:], AF.Ln)
    ldp = _st(c, "ldp")
    nc.vector.scalar_tensor_tensor(ldp[:], l1[:], 2.0, l2[:], OP.mult, OP.add)
    nc.vector.copy_predicated(y[:], c.outm[:], x_sp)
    insf = _st(c, "insf")
    nc.vector.tensor_tensor(insf[:], x_sp, xc[:], OP.is_equal)
    ldr = c.sb.tile([128, NS], F32R, tag="misc", name="ldr")
    nc.vector.tensor_tensor(ldr[:], ldp[:], insf[:], OP.mult)

    yr = yrl = yhb = None
    if need_yr:
        yr = c.sb.tile([128, NS], F32R, tag="rhbuf", name="yr")
        nc.scalar.copy(yr[:], y[:])
        yrl = c.sb.tile([128, NS], F32R, tag="rlbuf", name="yrl")
        nc.vector.tensor_tensor(yrl[:], y[:], yr[:], OP.subtract)
        yhb = c.sb.tile([128, NS], BF16, tag="hbbuf", name="yhb")
        nc.scalar.copy(yhb[:], yr[:])
    return yr, yrl, yhb, ldr


def build_nc():
    nc = bacc.Bacc(None)
    x_d = nc.declare_dram_parameter("x", [NCORE, DIM], F32, isOutput=False)
    wd = {}
    for cp in ("a", "b"):
        wd[f"w0h{cp}"] = nc.declare_dram_parameter(
            f"w0h{cp}", [128, HID], F32R, isOutput=False)
        wd[f"w0lb{cp}"] = nc.declare_dram_parameter(
            f"w0lb{cp}", [128, HID], BF16, isOutput=False)
        wd[f"w1h{cp}"] = nc.declare_dram_parameter(
            f"w1h{cp}", [128, 4, NPAR * 128], F32R, isOutput=False)
        wd[f"w1lb{cp}"] = nc.declare_dram_parameter(
            f"w1lb{cp}", [128, 4, NCOMP * 128], BF16, isOutput=False)
    ones_d = nc.declare_dram_parameter("ones", [128, 1], F32R, isOutput=False)
    z_d = nc.declare_dram_parameter("z", [NCORE, DIM], F32, isOutput=True)
    ld_d = nc.declare_dram_parameter("ld", [NCORE], F32, isOutput=True)

    c = Ctx()
    c.nc = nc

    with tile.TileContext(nc) as tc:
        with (
            tc.tile_pool(name="wpool", bufs=1) as wp,
            tc.tile_pool(name="sb", bufs=1) as sb,
            tc.tile_pool(name="ps_h", bufs=1, space="PSUM") as ps_h,
            tc.tile_pool(name="ps_par", bufs=1, space="PSUM") as ps_par,
            tc.tile_pool(name="ps_tr", bufs=1, space="PSUM") as ps_tr,
        ):
            c.sb, c.ps_h, c.ps_par, c.ps_tr = sb, ps_h, ps_par, ps_tr

            Ws = {}
            for cp in ("a", "b"):
                W = {}
                W["w0h"] = wp.tile([128, HID], F32R, tag=f"w0h{cp}",
                                   name=f"w0h{cp}")
                W["w0lb"] = wp.tile([128, HID], BF16, tag=f"w0lb{cp}",
                                    name=f"w0lb{cp}")
                W["w1h"] = wp.tile([128, 4, NPAR * 128], F32R,
                                   tag=f"w1h{cp}", name=f"w1h{cp}")
                W["w1lb"] = wp.tile([128, 4, NCOMP * 128], BF16,
                                    tag=f"w1lb{cp}", name=f"w1lb{cp}")
                for k2, t2 in W.items():
                    nc.sync.dma_start(t2[:], wd[f"{k2}{cp}"][:])
                Ws[cp] = W
            ones = wp.tile([128, 1], F32R)
            nc.sync.dma_start(ones[:], ones_d[:])
            ident = wp.tile([128, 128], F32)
            make_identity(nc, ident[:])
            c.consts = {}
            for cv in (2.0, BETA, B, 1.0, -B):
                ct = wp.tile([128, 1], F32, tag=f"const_{cv}",
                             name=f"const_{cv}")
                nc.gpsimd.memset(ct[:], cv)
                c.consts[cv] = ct
            c.S = wp.tile([128, 3, 5, NS], F32)
            nc.vector.memset(c.S[:, 0, 4, :], 1.0)
            nc.vector.memset(c.S[:, 1, 4, :], 1.0)
            nc.vector.memset(c.S[:, 2, 4, :], SP2_EDGE)
            c.msk = wp.tile([128, 4, NS], U8)
            c.outm = wp.tile([128, NS], U8)
            c.ll = {}
            for nm in ("idx", "delta", "t1m", "inh", "l2"):
                c.ll[nm] = wp.tile([128, NS], F32, tag=f"ll_{nm}",
                                   name=f"ll_{nm}")

            for it in range(NT):
                s0 = it * NS
                xn = sb.tile([128, 4, DIM], F32, tag="h_hb", name="xn")
                nc.sync.dma_start(
                    xn[:],
                    x_d[s0:s0 + NS, :].rearrange("(i p) c -> p i c", p=128))
                x0T = sb.tile([128, NS], F32, tag="x0T", name="x0T")
                x0Tr = sb.tile([128, NS], F32R, tag="rhbuf", name="x0Tr")
                x1T = sb.tile([128, NS], F32, tag="x1T", name="x1T")
                pst = ps_tr.tile([128, NS], F32, tag="pst", name="pst")
                for i in range(4):
                    nc.tensor.transpose(pst[:, i * 128:(i + 1) * 128],
                                        xn[:, i, 0:128], ident[:])
                nc.vector.tensor_copy(x0T[:], pst[:])
                nc.scalar.copy(x0Tr[:], pst[:])
                pst2 = ps_tr.tile([128, NS], F32, tag="pst", name="pst2")
                for i in range(4):
                    nc.tensor.transpose(pst2[:, i * 128:(i + 1) * 128],
                                        xn[:, i, 128:256], ident[:])
                nc.vector.tensor_copy(x1T[:], pst2[:])
                x0rl = sb.tile([128, NS], F32R, tag="rlbuf", name="x0rl")
                nc.vector.tensor_tensor(x0rl[:], x0T[:], x0Tr[:], OP.subtract)
                x0hb = sb.tile([128, NS], BF16, tag="hbbuf", name="x0hb")
                nc.scalar.copy(x0hb[:], x0Tr[:])

                psld = ps_h.tile([1, NS], F32, tag="psld", name="psld")

                y1 = sb.tile([128, NS], F32, tag="y1", name="y1")
                y1r, y1rl, y1hb, ldr1 = _emit_coupling(
                    c, Ws["a"], x1T[:], x0Tr[:], x0rl[:], x0hb[:],
                    y1, need_yr=True)
                nc.tensor.matmul(psld[:], ones[:], ldr1[:],
                                 start=True, stop=False)
                y0 = sb.tile([128, NS], F32, tag="x1T", name="y0")
                _, _, _, ldr0 = _emit_coupling(
                    c, Ws["b"], x0T[:], y1r[:], y1rl[:], y1hb[:],
                    y0, need_yr=False)
                nc.tensor.matmul(psld[:], ones[:], ldr0[:],
                                 start=False, stop=True)
                ldsb = sb.tile([1, NS], F32, tag="misc", name="ldsb")
                nc.scalar.copy(ldsb[:], psld[:])
                nc.sync.dma_start(ld_d[s0:s0 + NS].unsqueeze(0), ldsb[0:1, :])

                for half, yt in ((0, y0), (1, y1)):
                    psz = ps_tr.tile([128, NS], F32, tag="pst",
                                     name=f"psz{half}")
                    for i in range(4):
                        nc.tensor.transpose(psz[:, i * 128:(i + 1) * 128],
                                            yt[:, i * 128:(i + 1) * 128],
                                            ident[:])
                    zsb = sb.tile([128, NS], F32, tag="misc",
                                  name=f"zsb{half}")
                    nc.vector.tensor_copy(zsb[:], psz[:])
                    nc.sync.dma_start(
                        z_d[s0:s0 + NS, half * 128:(half + 1) * 128]
                            .rearrange("(i p) c -> p i c", p=128),
                        zsb[:].rearrange("p (i c) -> p i c", i=4))

    nc.finalize()
    return nc


_CACHED_NC = None


def _get_nc():
    global _CACHED_NC
    if _CACHED_NC is None:
        _CACHED_NC = build_nc()
    return _CACHED_NC


def _rnd12(a):
    bits = np.ascontiguousarray(a, np.float32).view(np.uint32)
    r = ((bits + np.uint32(0x7FF) + ((bits >> np.uint32(12)) & np.uint32(1)))
         & np.uint32(0xFFFFF000))
    return r.view(np.float32)


def _prep_weights(w0, w1):
    import ml_dtypes
    w0 = np.ascontiguousarray(w0, np.float32)
    w0h = _rnd12(w0)
    w0lb = (w0.astype(np.float64) - w0h).astype(ml_dtypes.bfloat16)
    # permute w1 columns (c,p interleaved -> param-major p*128+c)
    w1p = np.ascontiguousarray(
        w1.reshape(HID, HALF, NPAR).transpose(0, 2, 1).reshape(HID, NPAR * HALF))
    w1t = np.ascontiguousarray(
        w1p.reshape(4, 128, NPAR * HALF).transpose(1, 0, 2))
    w1h = _rnd12(w1t)
    w1lb = (w1t[:, :, :NCOMP * 128].astype(np.float64)
            - w1h[:, :, :NCOMP * 128]).astype(ml_dtypes.bfloat16)
    return (np.ascontiguousarray(w0h), np.ascontiguousarray(w0lb),
            np.ascontiguousarray(w1h), np.ascontiguousarray(w1lb))


def kernel(x, f0_w0, f0_b0, f0_w1, f0_b1, f1_w0, f1_b0, f1_w1, f1_b1):
    assert abs(f0_b0).max() == 0 and abs(f0_b1).max() == 0, \
        "bias folding assumes zero biases"
    assert abs(f1_b0).max() == 0 and abs(f1_b1).max() == 0, \
        "bias folding assumes zero biases"
    nc = _get_nc()
    wa = _prep_weights(np.asarray(f0_w0), np.asarray(f0_w1))
    wb = _prep_weights(np.asarray(f1_w0), np.asarray(f1_w1))
    ones = np.ones((128, 1), np.float32)
    x = np.asarray(x)
    in_maps = []
    for cc in range(NCORES):
        m = {"x": np.ascontiguousarray(x[cc * NCORE:(cc + 1) * NCORE]),
             "ones": ones}
        for cp, w4 in (("a", wa), ("b", wb)):
            m[f"w0h{cp}"], m[f"w0lb{cp}"], m[f"w1h{cp}"], m[f"w1lb{cp}"] = w4
        in_maps.append(m)
    res = run_bass_kernel_spmd(nc, in_maps, list(range(NCORES))).results
    z = np.concatenate([r["z"] for r in res], axis=0)
    ld = np.concatenate([r["ld"] for r in res], axis=0)
    return z, ld
